# revision 1
# baseline (speedup 1.0000x reference)
"""Trainium2 Bass kernel for nn_ChannelGate (pooling, complex channel attention).

Computation (per sample b):
  xr = x[b, :512], xi = x[b, 512:]            # [C, H*W]
  avg branch:  ar = mean(xr, hw), ai = mean(xi, hw)
  max branch:  score^2 = |z + 1/z|^2 = |z^2+1|^2 / |z|^2
               = ((d-1)^2 + (2 fr)^2) / d   with d = fr^2 + fi^2
               j* = argmax score^2; mr = fr[j*], mi = fi[j*]
  att = cMLP(ar, ai) + cMLP(mr, mi)           # tiny complex 2-layer MLP

Sharding: data-parallel over batch, 4 samples per core on 8 cores. The tiny
MLP weights are replicated; each core computes its own samples' outputs and
the host concatenates.
"""

import os

import numpy as np

_B, _C2, _H, _W = 32, 1024, 56, 56
_C = _C2 // 2
_HW = _H * _W
_NCORES = 8
_BLOC = _B // _NCORES  # samples per core
_KCH = _C // 128  # channel chunks of 128

_STATE = {}
last_results = None  # BassKernelResults of the most recent run (for test.py)


def _register_ops():
    """Register the fused custom DVE ops (idempotent per process)."""
    import concourse.dve_ops as dve_ops
    from concourse.dve_spec import C0, C1, One, Spec, Src0, Src1, maxx, sq
    from operator import add as op_add

    names = (
        "ANT_CG_SQSUM", "ANT_CG_CSCORE", "ANT_CG_MULMAX", "ANT_CG_MULSUM",
        "ANT_CG_FINDIDX",
    )
    if names[0] in dve_ops._SUB_OPCODE_FOR_NAME:
        by_name = {op.name: op for op in dve_ops.OPS}
        return {n: by_name[n] for n in names}

    # d = in0^2 + in1^2
    sq2_spec = Spec(
        body=sq(Src0) + sq(Src1),
        reference=lambda in0, in1, c0, c1, c2: (
            in0.astype(np.float32) ** 2 + in1.astype(np.float32) ** 2
        ),
    )
    # N = (in0 - 1)^2 + (c0 * in1)^2   (|z^2 + 1|^2 with in0 = |z|^2, in1 = Re z, c0 = 2)
    csc_spec = Spec(
        body=sq(Src0 - One) + sq(Src1 * C0),
        reference=lambda in0, in1, c0, c1, c2: (
            (in0.astype(np.float32) - 1.0) ** 2
            + (in1.astype(np.float32) * np.float32(c0)) ** 2
        ),
    )

    def _mul(in0, in1):
        return in0.astype(np.float32) * in1

    # out = in0*in1; accum = max(out)
    mulmax_spec = Spec(
        body=Src0 * Src1,
        accum=maxx,
        reference=lambda in0, in1, c0, c1, c2: (
            _mul(in0, in1),
            _mul(in0, in1).reshape(in0.shape[0], -1).max(axis=-1, keepdims=True),
        ),
    )
    # out = in0*in1; accum = sum(out)
    mulsum_spec = Spec(
        body=Src0 * Src1,
        accum=op_add,
        reference=lambda in0, in1, c0, c1, c2: (
            _mul(in0, in1),
            _mul(in0, in1).reshape(in0.shape[0], -1).sum(axis=-1, keepdims=True),
        ),
    )

    # accum = max over k of select(in0[k] == c0, c1 - k, 0)  → c1 - first argmatch
    from concourse.dve_spec import Idx, Zero, eq, select

    def _ref_findidx(in0, in1, c0, c1, c2):
        x = in0.astype(np.float32)
        n = x.reshape(x.shape[0], -1).shape[1]
        idxs = np.arange(n, dtype=np.float32)[None, :]
        body = np.where(x.reshape(x.shape[0], -1) == np.asarray(c0).reshape(-1, 1),
                        np.asarray(c1).reshape(-1, 1) - idxs, 0.0).astype(np.float32)
        return body.reshape(x.shape), body.max(axis=-1, keepdims=True)

    findidx_spec = Spec(
        body=select(eq(Src0, C0), C1 - Idx, Zero),
        accum=maxx,
        reference=_ref_findidx,
    )

    ops = {}
    for name, spec in zip(
        names, (sq2_spec, csc_spec, mulmax_spec, mulsum_spec, findidx_spec)
    ):
        op = dve_ops.DveOp(name, spec, subdim=False, uops_sha={})
        dve_ops.OPS.append(op)
        dve_ops.CUSTOM_DVE_SPECS[name] = spec
        dve_ops._SUB_OPCODE_FOR_NAME[name] = (
            max(dve_ops._SUB_OPCODE_FOR_NAME.values()) + 1
        )
        for ver in ("v3", "v4"):
            try:
                sha = dve_ops.DveOpSpec(
                    name=name,
                    opcode=dve_ops.get_dve_sub_opcode(name),
                    uops=dve_ops.lower(spec, ver=ver),
                    rd1_en=dve_ops.has_src1(spec),
                ).sha(ver)
                op.uops_sha[ver] = sha
            except Exception:
                pass
        ops[name] = op
    return ops


def _build_nc(repeat=1, variant="full"):
    ops = _register_ops()
    from contextlib import ExitStack

    import concourse.bacc as bacc
    import concourse.tile as tile
    from concourse import mybir

    f32 = mybir.dt.float32
    u16 = mybir.dt.uint16
    A = mybir.AluOpType
    AF = mybir.ActivationFunctionType
    SQ2 = ops["ANT_CG_SQSUM"]
    CSC = ops["ANT_CG_CSCORE"]
    MULMAX = ops["ANT_CG_MULMAX"]
    MULSUM = ops["ANT_CG_MULSUM"]
    FINDIDX = ops["ANT_CG_FINDIDX"]

    nc = bacc.Bacc("TRN2", target_bir_lowering=False, debug=False)
    x = nc.dram_tensor("x", [_BLOC, _C2, _HW], f32, kind="ExternalInput")
    w1rt = nc.dram_tensor("w1rt", [_C, 32], f32, kind="ExternalInput")
    w1it = nc.dram_tensor("w1it", [_C, 32], f32, kind="ExternalInput")
    w1itn = nc.dram_tensor("w1itn", [_C, 32], f32, kind="ExternalInput")
    w2rt = nc.dram_tensor("w2rt", [32, _C], f32, kind="ExternalInput")
    w2it = nc.dram_tensor("w2it", [32, _C], f32, kind="ExternalInput")
    w2itn = nc.dram_tensor("w2itn", [32, _C], f32, kind="ExternalInput")
    b1re = nc.dram_tensor("b1re", [32, 1], f32, kind="ExternalInput")
    b1im = nc.dram_tensor("b1im", [32, 1], f32, kind="ExternalInput")
    b2re2 = nc.dram_tensor("b2re2", [_KCH, 128], f32, kind="ExternalInput")
    b2im2 = nc.dram_tensor("b2im2", [_KCH, 128], f32, kind="ExternalInput")
    ident = nc.dram_tensor("ident", [128, 128], f32, kind="ExternalInput")
    dmask_r = nc.dram_tensor("dmask_r", [128, 32], f32, kind="ExternalInput")
    dmask_i = nc.dram_tensor("dmask_i", [128, 32], f32, kind="ExternalInput")
    out = nc.dram_tensor("out", [_BLOC, _C2], f32, kind="ExternalOutput")

    with ExitStack() as ctx:
        tc = ctx.enter_context(tile.TileContext(nc))
        singles = ctx.enter_context(tc.tile_pool(name="singles", bufs=1))
        work = ctx.enter_context(tc.tile_pool(name="work", bufs=2))
        small = ctx.enter_context(tc.tile_pool(name="small", bufs=2))
        mlp = ctx.enter_context(tc.tile_pool(name="mlp", bufs=1))
        psum = ctx.enter_context(tc.tile_pool(name="psum", bufs=2, space="PSUM"))

        # --- constants ---
        w1rt_t = singles.tile([128, _KCH, 32], f32)
        nc.gpsimd.dma_start(out=w1rt_t, in_=w1rt[:].rearrange("(k p) j -> p k j", p=128))
        w1it_t = singles.tile([128, _KCH, 32], f32)
        nc.gpsimd.dma_start(out=w1it_t, in_=w1it[:].rearrange("(k p) j -> p k j", p=128))
        w1itn_t = singles.tile([128, _KCH, 32], f32)
        nc.gpsimd.dma_start(
            out=w1itn_t, in_=w1itn[:].rearrange("(k p) j -> p k j", p=128)
        )
        w2rt_t = singles.tile([32, _C], f32)
        nc.gpsimd.dma_start(out=w2rt_t, in_=w2rt[:])
        w2it_t = singles.tile([32, _C], f32)
        nc.gpsimd.dma_start(out=w2it_t, in_=w2it[:])
        w2itn_t = singles.tile([32, _C], f32)
        nc.gpsimd.dma_start(out=w2itn_t, in_=w2itn[:])
        b1re_t = singles.tile([32, 1], f32)
        nc.gpsimd.dma_start(out=b1re_t, in_=b1re[:])
        b1im_t = singles.tile([32, 1], f32)
        nc.gpsimd.dma_start(out=b1im_t, in_=b1im[:])
        b2re2_t = singles.tile([128, _KCH], f32)
        nc.gpsimd.dma_start(out=b2re2_t, in_=b2re2[:].rearrange("k p -> p k"))
        b2im2_t = singles.tile([128, _KCH], f32)
        nc.gpsimd.dma_start(out=b2im2_t, in_=b2im2[:].rearrange("k p -> p k"))
        ident_t = singles.tile([128, 128], f32)
        nc.gpsimd.dma_start(out=ident_t, in_=ident[:])
        dmask_r_t = singles.tile([128, 32], f32)
        nc.gpsimd.dma_start(out=dmask_r_t, in_=dmask_r[:])
        dmask_i_t = singles.tile([128, 32], f32)
        nc.gpsimd.dma_start(out=dmask_i_t, in_=dmask_i[:])

        trash_a = singles.tile([128, _HW], f32)
        trash_b = singles.tile([128, _HW], f32)
        junk32 = singles.tile([128, 32], f32)
        # MLP inputs, transposed: [channel, sample-column]; cols 0-3 avg, 4-7 max
        stage_re = singles.tile([128, _KCH, 8], f32)
        stage_im = singles.tile([128, _KCH, 8], f32)
        # ACT-written means staging, merged into stage_* before the MLP so the
        # matmuls depend on a single writer engine.
        stage_avg_re = singles.tile([128, _KCH, 4], f32)
        stage_avg_im = singles.tile([128, _KCH, 4], f32)
        # Touch the mask constants on DVE once so the per-iteration ISA-encoded
        # DVE ops (1 wait slot only) never wait on these DMAs directly.
        nc.vector.tensor_copy(out=junk32, in_=dmask_r_t)
        nc.vector.tensor_copy(out=junk32, in_=dmask_i_t)

        xv = x[:]

        # Software pipeline: stage A (iter i): load + d + 1/d + N + means.
        # Stage B (emitted during iter i+1): score-max, argmax, gather.
        # Stage C (emitted during iter i+2): masked-reduce extraction.
        def emit_stage_b(st):
            if variant == "nomax":
                return None
            s2 = work.tile([128, _HW], f32, tag="s2")
            m2 = small.tile([128, 1], f32, tag="m2")
            nc.vector._custom_dve(MULMAX, out=s2, in0=st["nsc"], in1=st["y"], accum_out=m2)
            if variant == "noext":
                return None
            # acc = HW - argmax (first match); single fused pass, in-place out
            acc = small.tile([128, 1], f32, tag="acc")
            nc.vector._custom_dve(
                FINDIDX, out=s2, in0=s2, s0=m2, s1=float(_HW), accum_out=acc
            )
            if variant == "nofind":
                return None
            # gather winners: per 16-partition group, fetch all 16 indices;
            # the (p, p%16) diagonal is extracted in stage C.
            # idx2 = [HW - acc, 2*HW - acc] as uint16
            idx2 = small.tile([128, 2], u16, tag="idx2")
            nc.vector.tensor_scalar(
                out=idx2[:, 0:1], in0=acc, scalar1=-1.0, scalar2=float(_HW),
                op0=A.mult, op1=A.add,
            )
            nc.vector.tensor_scalar(
                out=idx2[:, 1:2], in0=acc, scalar1=-1.0, scalar2=float(2 * _HW),
                op0=A.mult, op1=A.add,
            )
            if variant == "noicopy":
                return None
            gath = small.tile([128, 32], f32, tag="gath")
            nc.gpsimd.indirect_copy(
                out=gath, data=st["X"][:].rearrange("p a b -> p (a b)"), idxs=idx2,
                i_know_ap_gather_is_preferred=True,
            )
            if variant == "noc":
                return None
            return {"gath": gath, "k": st["k"], "b": st["b"]}

        def emit_stage_c(st):
            nc.vector._custom_dve(
                MULSUM, out=junk32, in0=st["gath"], in1=dmask_r_t,
                accum_out=stage_re[:, st["k"], 4 + st["b"] : 5 + st["b"]],
            )
            nc.vector._custom_dve(
                MULSUM, out=junk32, in0=st["gath"], in1=dmask_i_t,
                accum_out=stage_im[:, st["k"], 4 + st["b"] : 5 + st["b"]],
            )

        prev1 = None
        prev2 = None
        for b, k in [(b, k) for _ in range(repeat)
                     for b in range(_BLOC) for k in range(_KCH)]:
                X = work.tile([128, 2, _HW], f32, tag="X")
                # one DMA for both halves (real chunk k, imag chunk k); issued
                # on SP HWDGE so gpsimd only runs the gathers (Bacc splits any
                # multi-queue waits into event-semaphore chains)
                src = xv[b].rearrange("(j c) w -> c j w", j=2)[k * 128 : (k + 1) * 128]
                nc.sync.dma_start(out=X, in_=src)
                fr = X[:, 0, :]
                fi = X[:, 1, :]

                d = work.tile([128, _HW], f32, tag="d")
                nc.vector._custom_dve(SQ2, out=d, in0=fr, in1=fi)
                # channel means on ACT first (no DVE dependency) so ACT never
                # stalls waiting for d at iteration boundaries
                nc.scalar.activation(
                    out=trash_a, in_=fr, func=AF.Copy, bias=0.0, scale=1.0 / _HW,
                    accum_out=stage_avg_re[:, k, b : b + 1],
                )
                nc.scalar.activation(
                    out=trash_b, in_=fi, func=AF.Copy, bias=0.0, scale=1.0 / _HW,
                    accum_out=stage_avg_im[:, k, b : b + 1],
                )
                # y = 1/d on ACT via exp(-ln d); ln+exp live in one table set
                y = work.tile([128, _HW], f32, tag="y")
                nc.scalar.activation(out=y, in_=d, func=AF.Ln)
                nc.scalar.activation(out=y, in_=y, func=AF.Exp, scale=-1.0)
                nsc = work.tile([128, _HW], f32, tag="nsc")
                nc.vector._custom_dve(CSC, out=nsc, in0=d, in1=fr, s0=2.0)

                nxt2 = emit_stage_b(prev1) if prev1 is not None else None
                if prev2 is not None:
                    emit_stage_c(prev2)
                prev2 = nxt2
                prev1 = {"nsc": nsc, "y": y, "X": X, "k": k, "b": b}
        # drain the pipeline
        nxt2 = emit_stage_b(prev1)
        if prev2 is not None:
            emit_stage_c(prev2)
        if nxt2 is not None:
            emit_stage_c(nxt2)

        # --- tiny complex MLP on PE (transposed layout [feature, column]) ---
        nc.vector.tensor_copy(out=stage_re[:, :, 0:4], in_=stage_avg_re)
        nc.vector.tensor_copy(out=stage_im[:, :, 0:4], in_=stage_avg_im)
        hps = psum.tile([32, 2, 8], f32, tag="hps")
        for k in range(_KCH):
            nc.tensor.matmul(
                hps[:, 0, :], lhsT=w1rt_t[:, k, :], rhs=stage_re[:, k, :],
                start=(k == 0), stop=False,
            )
        for k in range(_KCH):
            nc.tensor.matmul(
                hps[:, 0, :], lhsT=w1itn_t[:, k, :], rhs=stage_im[:, k, :],
                start=False, stop=(k == _KCH - 1),
            )
        for k in range(_KCH):
            nc.tensor.matmul(
                hps[:, 1, :], lhsT=w1rt_t[:, k, :], rhs=stage_im[:, k, :],
                start=(k == 0), stop=False,
            )
        for k in range(_KCH):
            nc.tensor.matmul(
                hps[:, 1, :], lhsT=w1it_t[:, k, :], rhs=stage_re[:, k, :],
                start=False, stop=(k == _KCH - 1),
            )
        hreT = mlp.tile([32, 8], f32)
        nc.vector.tensor_scalar(
            out=hreT, in0=hps[:, 0, :], scalar1=b1re_t, scalar2=None, op0=A.add
        )
        himT = mlp.tile([32, 8], f32)
        nc.vector.tensor_scalar(
            out=himT, in0=hps[:, 1, :], scalar1=b1im_t, scalar2=None, op0=A.add
        )

        # cardioid: s = 0.5 * (1 + re / |h|)
        q2 = mlp.tile([32, 8], f32)
        nc.vector._custom_dve(SQ2, out=q2, in0=hreT, in1=himT)
        ah = mlp.tile([32, 8], f32)
        nc.scalar.activation(out=ah, in_=q2, func=AF.Sqrt)
        rh = mlp.tile([32, 8], f32)
        nc.vector.reciprocal(out=rh, in_=ah)
        s = mlp.tile([32, 8], f32)
        nc.vector.tensor_tensor(out=s, in0=hreT, in1=rh, op=A.mult)
        nc.vector.tensor_scalar(out=s, in0=s, scalar1=0.5, scalar2=0.5, op0=A.mult, op1=A.add)
        greT = mlp.tile([32, 8], f32)
        nc.vector.tensor_tensor(out=greT, in0=hreT, in1=s, op=A.mult)
        gimT = mlp.tile([32, 8], f32)
        nc.vector.tensor_tensor(out=gimT, in0=himT, in1=s, op=A.mult)

        out_sb = singles.tile([_BLOC, _C2], f32)
        for m in range(_KCH):
            sl = slice(m * 128, (m + 1) * 128)
            ore = psum.tile([128, 8], f32, tag="ore")
            nc.tensor.matmul(ore, lhsT=w2rt_t[:, sl], rhs=greT, start=True, stop=False)
            nc.tensor.matmul(ore, lhsT=w2itn_t[:, sl], rhs=gimT, start=False, stop=True)
            osb_re = mlp.tile([128, 8], f32, tag="osb")
            nc.scalar.copy(out=osb_re, in_=ore)
            fre = mlp.tile([128, 4], f32, tag="fre")
            nc.vector.tensor_tensor(out=fre, in0=osb_re[:, 0:4], in1=osb_re[:, 4:8], op=A.add)
            nc.vector.tensor_scalar(
                out=fre, in0=fre, scalar1=b2re2_t[:, m : m + 1], scalar2=None, op0=A.add
            )
            tps = psum.tile([4, 128], f32, tag="tps")
            nc.tensor.transpose(tps, fre, ident_t)
            nc.vector.tensor_copy(out=out_sb[:, sl], in_=tps)

            oim = psum.tile([128, 8], f32, tag="oim")
            nc.tensor.matmul(oim, lhsT=w2it_t[:, sl], rhs=greT, start=True, stop=False)
            nc.tensor.matmul(oim, lhsT=w2rt_t[:, sl], rhs=gimT, start=False, stop=True)
            osb_im = mlp.tile([128, 8], f32, tag="osb")
            nc.scalar.copy(out=osb_im, in_=oim)
            fim = mlp.tile([128, 4], f32, tag="fim")
            nc.vector.tensor_tensor(out=fim, in0=osb_im[:, 0:4], in1=osb_im[:, 4:8], op=A.add)
            nc.vector.tensor_scalar(
                out=fim, in0=fim, scalar1=b2im2_t[:, m : m + 1], scalar2=None, op0=A.add
            )
            tps2 = psum.tile([4, 128], f32, tag="tps")
            nc.tensor.transpose(tps2, fim, ident_t)
            nc.vector.tensor_copy(out=out_sb[:, _C + m * 128 : _C + (m + 1) * 128], in_=tps2)

        nc.gpsimd.dma_start(out=out[:], in_=out_sb)

    nc.compile()
    return nc


def _host_inputs(w1r, b1r, w1i, b1i, w2r, b2r, w2i, b2i):
    f32 = np.float32
    shared = {
        "w1rt": np.ascontiguousarray(w1r.T, dtype=f32),
        "w1it": np.ascontiguousarray(w1i.T, dtype=f32),
        "w1itn": np.ascontiguousarray(-w1i.T, dtype=f32),
        "w2rt": np.ascontiguousarray(w2r.T, dtype=f32),
        "w2it": np.ascontiguousarray(w2i.T, dtype=f32),
        "w2itn": np.ascontiguousarray(-w2i.T, dtype=f32),
        "b1re": np.ascontiguousarray((b1r - b1i).reshape(32, 1), dtype=f32),
        "b1im": np.ascontiguousarray((b1r + b1i).reshape(32, 1), dtype=f32),
        "b2re2": np.ascontiguousarray((2.0 * (b2r - b2i)).reshape(_KCH, 128), dtype=f32),
        "b2im2": np.ascontiguousarray((2.0 * (b2r + b2i)).reshape(_KCH, 128), dtype=f32),
        "ident": np.eye(128, dtype=f32),
    }
    p = np.arange(128) % 16
    dm_r = np.zeros((128, 32), dtype=f32)
    dm_r[np.arange(128), p] = 1.0
    dm_i = np.zeros((128, 32), dtype=f32)
    dm_i[np.arange(128), 16 + p] = 1.0
    shared["dmask_r"] = dm_r
    shared["dmask_i"] = dm_i
    return shared


def kernel(x, w1r, b1r, w1i, b1i, w2r, b2r, w2i, b2i):
    global last_results
    from concourse.bass_utils import run_bass_kernel_spmd

    x = np.ascontiguousarray(np.asarray(x), dtype=np.float32)
    args = [np.asarray(a, dtype=np.float32) for a in (w1r, b1r, w1i, b1i, w2r, b2r, w2i, b2i)]
    w1r, b1r, w1i, b1i, w2r, b2r, w2i, b2i = args

    if "nc" not in _STATE:
        _STATE["nc"] = _build_nc()
    nc = _STATE["nc"]

    shared = _host_inputs(w1r, b1r, w1i, b1i, w2r, b2r, w2i, b2i)
    xr3 = x.reshape(_B, _C2, _HW)
    in_maps = []
    for i in range(_NCORES):
        m = dict(shared)
        m["x"] = np.ascontiguousarray(xr3[i * _BLOC : (i + 1) * _BLOC])
        in_maps.append(m)

    trace = os.environ.get("KERNEL_TRACE", "0") == "1"
    res = run_bass_kernel_spmd(nc, in_maps, core_ids=list(range(_NCORES)), trace=trace)
    last_results = res
    return np.concatenate([r["out"] for r in res.results], axis=0)



# revision 2
# speedup vs baseline: 1.2191x; 1.2191x over previous
"""Trainium2 Bass kernel for nn_ChannelGate (pooling, complex channel attention).

Computation (per sample b):
  xr = x[b, :512], xi = x[b, 512:]            # [C, H*W]
  avg branch:  ar = mean(xr, hw), ai = mean(xi, hw)
  max branch:  score^2 = |z + 1/z|^2 = |z^2+1|^2 / |z|^2
               = ((d-1)^2 + (2 fr)^2) / d   with d = fr^2 + fi^2
               j* = argmax score^2; mr = fr[j*], mi = fi[j*]
  att = cMLP(ar, ai) + cMLP(mr, mi)           # tiny complex 2-layer MLP

Sharding: data-parallel over batch, 4 samples per core on 8 cores. The tiny
MLP weights are replicated; each core computes its own samples' outputs and
the host concatenates.

Engine budget per (b, k) tile [128 ch, 3136 hw] (the kernel is a 3-way tie
between DVE / ACT / DMA near the HBM roofline):
  DVE  3 full passes: d (+ a "spike" that deposits running-sum(fr) into a
       pad column via scan/select), nsc, and a fused score*argmax pass
       (s = nsc*y; emit Idx where s equals its running max; accum MAX
       returns the argmax directly — replaces the old mulmax+findidx pair).
  ACT  3 full passes: Ln(d), Exp(-ln) = 1/d, and Copy(fi)+accum = mean(fi).
       Activation-table thrash (Ln->natural_log, Exp->exp_and_others, 1.3us
       per swap, 2 per iter) is eliminated by restricting the table map so
       both resolve to natural_log_exp_and_others; the fixpoint pass then
       hoists the single load out of the loop.
  DMA  one 3.2MB load (the roofline term).
"""

import os

import numpy as np

_B, _C2, _H, _W = 32, 1024, 56, 56
_C = _C2 // 2
_HW = _H * _W
_HWP = _HW + 1  # spatial extent + 1 pad column for the mean-sum spike
_NCORES = 8
_BLOC = _B // _NCORES  # samples per core
_KCH = _C // 128  # channel chunks of 128

_STATE = {}
last_results = None  # BassKernelResults of the most recent run (for test.py)


def _register_ops():
    """Register the fused custom DVE ops (idempotent per process)."""
    import concourse.dve_ops as dve_ops
    from concourse.dve_spec import (
        AluOp, C0, Idx, One, Spec, Src0, Src1, Zero, eq, maxx, scan, select, sq,
    )
    from operator import add as op_add

    names = ("ANT_CG_SQ2SPK", "ANT_CG_CSCORE", "ANT_CG_ARGMAX", "ANT_CG_MULSUM")
    if names[0] in dve_ops._SUB_OPCODE_FOR_NAME:
        by_name = {op.name: op for op in dve_ops.OPS}
        return {n: by_name[n] for n in names}

    def _c_int(c):
        return int(np.asarray(c).reshape(-1)[0])

    # d = in0^2 + in1^2, except at stream position c0 where the running
    # sum of in0 (inclusive prefix) is emitted instead. With a zeroed pad
    # column at position c0 this deposits sum(fr) into d[:, c0].
    def _ref_sq2spk(in0, in1, c0, c1, c2):
        x0 = in0.astype(np.float32)
        x1 = in1.astype(np.float32)
        body = x0 * x0 + x1 * x1
        k = _c_int(c0)
        cs = np.cumsum(x0, axis=-1, dtype=np.float32)
        body[..., k] = cs[..., k]
        return body

    sq2spk_spec = Spec(
        body=select(eq(Idx, C0), scan(AluOp.ADD, Src0), sq(Src0) + sq(Src1)),
        reference=_ref_sq2spk,
    )

    # N = (in0 - 1)^2 + (c0 * in1)^2   (|z^2 + 1|^2 with in0 = |z|^2, in1 = Re z, c0 = 2)
    csc_spec = Spec(
        body=sq(Src0 - One) + sq(Src1 * C0),
        reference=lambda in0, in1, c0, c1, c2: (
            (in0.astype(np.float32) - 1.0) ** 2
            + (in1.astype(np.float32) * np.float32(c0)) ** 2
        ),
    )

    # s = in0*in1; body emits Idx where s equals its running max (prefix-max
    # positions), else 0; accum MAX of the body is the argmax of s (last
    # occurrence on exact float ties — measure-zero for this input).
    def _ref_argmax(in0, in1, c0, c1, c2):
        s = in0.astype(np.float32) * in1.astype(np.float32)
        m = np.maximum.accumulate(s, axis=-1)
        idxs = np.arange(s.shape[-1], dtype=np.float32)
        body = np.where(s == m, idxs, 0.0).astype(np.float32)
        return body, body.max(axis=-1, keepdims=True)

    _s = Src0 * Src1
    argmax_spec = Spec(
        body=select(eq(_s, scan(AluOp.MAX, _s)), Idx, Zero),
        accum=maxx,
        reference=_ref_argmax,
    )

    def _mul(in0, in1):
        return in0.astype(np.float32) * in1

    # out = in0*in1; accum = sum(out)
    mulsum_spec = Spec(
        body=Src0 * Src1,
        accum=op_add,
        reference=lambda in0, in1, c0, c1, c2: (
            _mul(in0, in1),
            _mul(in0, in1).reshape(in0.shape[0], -1).sum(axis=-1, keepdims=True),
        ),
    )

    ops = {}
    for name, spec in zip(
        names, (sq2spk_spec, csc_spec, argmax_spec, mulsum_spec)
    ):
        op = dve_ops.DveOp(name, spec, subdim=False, uops_sha={})
        dve_ops.OPS.append(op)
        dve_ops.CUSTOM_DVE_SPECS[name] = spec
        dve_ops._SUB_OPCODE_FOR_NAME[name] = (
            max(dve_ops._SUB_OPCODE_FOR_NAME.values()) + 1
        )
        for ver in ("v3", "v4"):
            try:
                sha = dve_ops.DveOpSpec(
                    name=name,
                    opcode=dve_ops.get_dve_sub_opcode(name),
                    uops=dve_ops.lower(spec, ver=ver),
                    rd1_en=dve_ops.has_src1(spec),
                ).sha(ver)
                op.uops_sha[ver] = sha
            except Exception:
                pass
        ops[name] = op
    return ops


def _patch_act_tables():
    """Pin Ln and Exp to the one table set containing both.

    The table-load placement pass assigns each activation the FIRST set
    containing its function (Ln -> natural_log, Exp -> exp_and_others),
    which costs two 1.3us ACT_TABLE_LOADs per loop iteration. Removing
    ln/exp from every other set (indices untouched) makes both resolve to
    natural_log_exp_and_others, and the fixpoint hoists the load out of
    the loop entirely.
    """
    import concourse.bacc as bacc_mod
    from concourse import mybir

    AF = mybir.ActivationFunctionType
    orig = bacc_mod.get_activation_tables
    if getattr(orig, "_ant_cg_patched", False):
        return
    def patched(arch):
        t = {}
        for name, funcs in orig(arch).items():
            funcs = set(funcs)
            if name != "natural_log_exp_and_others":
                funcs.discard(AF.Ln)
                funcs.discard(AF.Exp)
            t[name] = funcs
        return t
    patched._ant_cg_patched = True
    bacc_mod.get_activation_tables = patched


def _build_nc(repeat=1):
    ops = _register_ops()
    _patch_act_tables()
    from contextlib import ExitStack

    import concourse.bacc as bacc
    import concourse.tile as tile
    from concourse import mybir

    f32 = mybir.dt.float32
    u16 = mybir.dt.uint16
    A = mybir.AluOpType
    AF = mybir.ActivationFunctionType
    SQ2SPK = ops["ANT_CG_SQ2SPK"]
    CSC = ops["ANT_CG_CSCORE"]
    ARGMAX = ops["ANT_CG_ARGMAX"]
    MULSUM = ops["ANT_CG_MULSUM"]

    nc = bacc.Bacc("TRN2", target_bir_lowering=False, debug=False)
    x = nc.dram_tensor("x", [_BLOC, _C2, _HW], f32, kind="ExternalInput")
    w1rt = nc.dram_tensor("w1rt", [_C, 32], f32, kind="ExternalInput")
    w1it = nc.dram_tensor("w1it", [_C, 32], f32, kind="ExternalInput")
    w1itn = nc.dram_tensor("w1itn", [_C, 32], f32, kind="ExternalInput")
    w2rt = nc.dram_tensor("w2rt", [32, _C], f32, kind="ExternalInput")
    w2it = nc.dram_tensor("w2it", [32, _C], f32, kind="ExternalInput")
    w2itn = nc.dram_tensor("w2itn", [32, _C], f32, kind="ExternalInput")
    b1re = nc.dram_tensor("b1re", [32, 1], f32, kind="ExternalInput")
    b1im = nc.dram_tensor("b1im", [32, 1], f32, kind="ExternalInput")
    b2re2 = nc.dram_tensor("b2re2", [_KCH, 128], f32, kind="ExternalInput")
    b2im2 = nc.dram_tensor("b2im2", [_KCH, 128], f32, kind="ExternalInput")
    ident = nc.dram_tensor("ident", [128, 128], f32, kind="ExternalInput")
    dmask_r = nc.dram_tensor("dmask_r", [128, 32], f32, kind="ExternalInput")
    dmask_i = nc.dram_tensor("dmask_i", [128, 32], f32, kind="ExternalInput")
    out = nc.dram_tensor("out", [_BLOC, _C2], f32, kind="ExternalOutput")

    with ExitStack() as ctx:
        tc = ctx.enter_context(tile.TileContext(nc))
        singles = ctx.enter_context(tc.tile_pool(name="singles", bufs=1))
        work = ctx.enter_context(tc.tile_pool(name="work", bufs=2))
        small = ctx.enter_context(tc.tile_pool(name="small", bufs=2))
        mlp = ctx.enter_context(tc.tile_pool(name="mlp", bufs=1))
        psum = ctx.enter_context(tc.tile_pool(name="psum", bufs=2, space="PSUM"))

        # --- constants ---
        w1rt_t = singles.tile([128, _KCH, 32], f32)
        nc.gpsimd.dma_start(out=w1rt_t, in_=w1rt[:].rearrange("(k p) j -> p k j", p=128))
        w1it_t = singles.tile([128, _KCH, 32], f32)
        nc.gpsimd.dma_start(out=w1it_t, in_=w1it[:].rearrange("(k p) j -> p k j", p=128))
        w1itn_t = singles.tile([128, _KCH, 32], f32)
        nc.gpsimd.dma_start(
            out=w1itn_t, in_=w1itn[:].rearrange("(k p) j -> p k j", p=128)
        )
        w2rt_t = singles.tile([32, _C], f32)
        nc.gpsimd.dma_start(out=w2rt_t, in_=w2rt[:])
        w2it_t = singles.tile([32, _C], f32)
        nc.gpsimd.dma_start(out=w2it_t, in_=w2it[:])
        w2itn_t = singles.tile([32, _C], f32)
        nc.gpsimd.dma_start(out=w2itn_t, in_=w2itn[:])
        b1re_t = singles.tile([32, 1], f32)
        nc.gpsimd.dma_start(out=b1re_t, in_=b1re[:])
        b1im_t = singles.tile([32, 1], f32)
        nc.gpsimd.dma_start(out=b1im_t, in_=b1im[:])
        b2re2_t = singles.tile([128, _KCH], f32)
        nc.gpsimd.dma_start(out=b2re2_t, in_=b2re2[:].rearrange("k p -> p k"))
        b2im2_t = singles.tile([128, _KCH], f32)
        nc.gpsimd.dma_start(out=b2im2_t, in_=b2im2[:].rearrange("k p -> p k"))
        ident_t = singles.tile([128, 128], f32)
        nc.gpsimd.dma_start(out=ident_t, in_=ident[:])
        dmask_r_t = singles.tile([128, 32], f32)
        nc.gpsimd.dma_start(out=dmask_r_t, in_=dmask_r[:])
        dmask_i_t = singles.tile([128, 32], f32)
        nc.gpsimd.dma_start(out=dmask_i_t, in_=dmask_i[:])

        junk_full = singles.tile([128, _HW], f32)  # argmax body output (unused)
        trash_a = singles.tile([128, _HW], f32)  # ACT mean-pass output (unused)
        junk32 = singles.tile([128, 32], f32)
        # MLP inputs, transposed: [channel, sample-column]; cols 0-3 avg, 4-7 max
        stage_re = singles.tile([128, _KCH, 8], f32)
        stage_im = singles.tile([128, _KCH, 8], f32)
        # means staging, merged into stage_* before the MLP so the matmuls
        # depend on a single writer engine.
        stage_avg_re = singles.tile([128, _KCH, 4], f32)
        stage_avg_im = singles.tile([128, _KCH, 4], f32)
        # Touch the mask constants on DVE once so the per-iteration ISA-encoded
        # DVE ops (1 wait slot only) never wait on these DMAs directly.
        nc.vector.tensor_copy(out=junk32, in_=dmask_r_t)
        nc.vector.tensor_copy(out=junk32, in_=dmask_i_t)

        xv = x[:]

        # Software pipeline: stage A (iter i): load + d/spike + 1/d + N +
        # means. Stage B (emitted during iter i+1): fused argmax, gather.
        # Stage C (emitted during iter i+2): masked-reduce extraction.
        def emit_stage_b(st):
            # acc = argmax_j (nsc[j] * y[j]) as f32, single fused pass
            acc = small.tile([128, 1], f32, tag="acc")
            nc.vector._custom_dve(
                ARGMAX, out=junk_full, in0=st["nsc"], in1=st["y"], accum_out=acc
            )
            # gather winners: per 16-partition group, fetch all 16 indices;
            # the (p, p%16) diagonal is extracted in stage C.
            # idx2 = [j, HWP + j] as uint16 (fi half starts at offset HWP)
            idx2 = small.tile([128, 2], u16, tag="idx2")
            nc.vector.tensor_scalar(
                out=idx2[:, 0:1], in0=acc, scalar1=1.0, scalar2=0.0,
                op0=A.mult, op1=A.add,
            )
            nc.vector.tensor_scalar(
                out=idx2[:, 1:2], in0=acc, scalar1=1.0, scalar2=float(_HWP),
                op0=A.mult, op1=A.add,
            )
            gath = small.tile([128, 32], f32, tag="gath")
            nc.gpsimd.indirect_copy(
                out=gath, data=st["X"][:].rearrange("p a b -> p (a b)"), idxs=idx2,
                i_know_ap_gather_is_preferred=True,
            )
            return {"gath": gath, "k": st["k"], "b": st["b"]}

        def emit_stage_c(st):
            nc.vector._custom_dve(
                MULSUM, out=junk32, in0=st["gath"], in1=dmask_r_t,
                accum_out=stage_re[:, st["k"], 4 + st["b"] : 5 + st["b"]],
            )
            nc.vector._custom_dve(
                MULSUM, out=junk32, in0=st["gath"], in1=dmask_i_t,
                accum_out=stage_im[:, st["k"], 4 + st["b"] : 5 + st["b"]],
            )

        prev1 = None
        prev2 = None
        for b, k in [(b, k) for _ in range(repeat)
                     for b in range(_BLOC) for k in range(_KCH)]:
                X = work.tile([128, 2, _HWP], f32, tag="X")
                # zero the pad column (both halves) so the fr prefix-sum
                # spike is exact and the pad never wins anything
                nc.gpsimd.memset(X[:, :, _HW:_HWP], 0.0)
                # one DMA for both halves (real chunk k, imag chunk k); issued
                # on SP HWDGE so gpsimd only runs the gathers (Bacc splits any
                # multi-queue waits into event-semaphore chains)
                src = xv[b].rearrange("(j c) w -> c j w", j=2)[k * 128 : (k + 1) * 128]
                nc.sync.dma_start(out=X[:, :, 0:_HW], in_=src)
                fr = X[:, 0, :]
                fi = X[:, 1, :]

                # d = fr^2 + fi^2 over [0:HW); d[HW] = sum(fr) via the spike
                d = work.tile([128, _HWP], f32, tag="d")
                nc.vector._custom_dve(
                    SQ2SPK, out=d, in0=fr, in1=fi, s0=float(_HW)
                )
                # mean(fi) on ACT (independent of the DVE chain)
                nc.scalar.activation(
                    out=trash_a, in_=X[:, 1, 0:_HW], func=AF.Copy, bias=0.0,
                    scale=1.0 / _HW,
                    accum_out=stage_avg_im[:, k, b : b + 1],
                )
                # y = 1/d on ACT via exp(-ln d); both live in one table set
                y = work.tile([128, _HW], f32, tag="y")
                nc.scalar.activation(out=y, in_=d[:, 0:_HW], func=AF.Ln)
                nc.scalar.activation(out=y, in_=y, func=AF.Exp, scale=-1.0)
                nsc = work.tile([128, _HW], f32, tag="nsc")
                nc.vector._custom_dve(
                    CSC, out=nsc, in0=d[:, 0:_HW], in1=X[:, 0, 0:_HW], s0=2.0
                )
                # mean(fr) = spike / HW
                nc.vector.tensor_scalar(
                    out=stage_avg_re[:, k, b : b + 1], in0=d[:, _HW:_HWP],
                    scalar1=1.0 / _HW, scalar2=0.0, op0=A.mult, op1=A.add,
                )

                nxt2 = emit_stage_b(prev1) if prev1 is not None else None
                if prev2 is not None:
                    emit_stage_c(prev2)
                prev2 = nxt2
                prev1 = {"nsc": nsc, "y": y, "X": X, "k": k, "b": b}
        # drain the pipeline
        nxt2 = emit_stage_b(prev1)
        if prev2 is not None:
            emit_stage_c(prev2)
        if nxt2 is not None:
            emit_stage_c(nxt2)

        # --- tiny complex MLP on PE (transposed layout [feature, column]) ---
        nc.vector.tensor_copy(out=stage_re[:, :, 0:4], in_=stage_avg_re)
        nc.vector.tensor_copy(out=stage_im[:, :, 0:4], in_=stage_avg_im)
        hps = psum.tile([32, 2, 8], f32, tag="hps")
        for k in range(_KCH):
            nc.tensor.matmul(
                hps[:, 0, :], lhsT=w1rt_t[:, k, :], rhs=stage_re[:, k, :],
                start=(k == 0), stop=False,
            )
        for k in range(_KCH):
            nc.tensor.matmul(
                hps[:, 0, :], lhsT=w1itn_t[:, k, :], rhs=stage_im[:, k, :],
                start=False, stop=(k == _KCH - 1),
            )
        for k in range(_KCH):
            nc.tensor.matmul(
                hps[:, 1, :], lhsT=w1rt_t[:, k, :], rhs=stage_im[:, k, :],
                start=(k == 0), stop=False,
            )
        for k in range(_KCH):
            nc.tensor.matmul(
                hps[:, 1, :], lhsT=w1it_t[:, k, :], rhs=stage_re[:, k, :],
                start=False, stop=(k == _KCH - 1),
            )
        hreT = mlp.tile([32, 8], f32)
        nc.vector.tensor_scalar(
            out=hreT, in0=hps[:, 0, :], scalar1=b1re_t, scalar2=None, op0=A.add
        )
        himT = mlp.tile([32, 8], f32)
        nc.vector.tensor_scalar(
            out=himT, in0=hps[:, 1, :], scalar1=b1im_t, scalar2=None, op0=A.add
        )

        # cardioid: s = 0.5 * (1 + re / |h|)
        q2 = mlp.tile([32, 8], f32)
        nc.vector.tensor_tensor(out=q2, in0=hreT, in1=hreT, op=A.mult)
        q2b = mlp.tile([32, 8], f32)
        nc.vector.tensor_tensor(out=q2b, in0=himT, in1=himT, op=A.mult)
        nc.vector.tensor_tensor(out=q2, in0=q2, in1=q2b, op=A.add)
        ah = mlp.tile([32, 8], f32)
        nc.scalar.activation(out=ah, in_=q2, func=AF.Sqrt)
        rh = mlp.tile([32, 8], f32)
        nc.vector.reciprocal(out=rh, in_=ah)
        s = mlp.tile([32, 8], f32)
        nc.vector.tensor_tensor(out=s, in0=hreT, in1=rh, op=A.mult)
        nc.vector.tensor_scalar(out=s, in0=s, scalar1=0.5, scalar2=0.5, op0=A.mult, op1=A.add)
        greT = mlp.tile([32, 8], f32)
        nc.vector.tensor_tensor(out=greT, in0=hreT, in1=s, op=A.mult)
        gimT = mlp.tile([32, 8], f32)
        nc.vector.tensor_tensor(out=gimT, in0=himT, in1=s, op=A.mult)

        out_sb = singles.tile([_BLOC, _C2], f32)
        for m in range(_KCH):
            sl = slice(m * 128, (m + 1) * 128)
            ore = psum.tile([128, 8], f32, tag="ore")
            nc.tensor.matmul(ore, lhsT=w2rt_t[:, sl], rhs=greT, start=True, stop=False)
            nc.tensor.matmul(ore, lhsT=w2itn_t[:, sl], rhs=gimT, start=False, stop=True)
            osb_re = mlp.tile([128, 8], f32, tag="osb")
            nc.scalar.copy(out=osb_re, in_=ore)
            fre = mlp.tile([128, 4], f32, tag="fre")
            nc.vector.tensor_tensor(out=fre, in0=osb_re[:, 0:4], in1=osb_re[:, 4:8], op=A.add)
            nc.vector.tensor_scalar(
                out=fre, in0=fre, scalar1=b2re2_t[:, m : m + 1], scalar2=None, op0=A.add
            )
            tps = psum.tile([4, 128], f32, tag="tps")
            nc.tensor.transpose(tps, fre, ident_t)
            nc.vector.tensor_copy(out=out_sb[:, sl], in_=tps)

            oim = psum.tile([128, 8], f32, tag="oim")
            nc.tensor.matmul(oim, lhsT=w2it_t[:, sl], rhs=greT, start=True, stop=False)
            nc.tensor.matmul(oim, lhsT=w2rt_t[:, sl], rhs=gimT, start=False, stop=True)
            osb_im = mlp.tile([128, 8], f32, tag="osb")
            nc.scalar.copy(out=osb_im, in_=oim)
            fim = mlp.tile([128, 4], f32, tag="fim")
            nc.vector.tensor_tensor(out=fim, in0=osb_im[:, 0:4], in1=osb_im[:, 4:8], op=A.add)
            nc.vector.tensor_scalar(
                out=fim, in0=fim, scalar1=b2im2_t[:, m : m + 1], scalar2=None, op0=A.add
            )
            tps2 = psum.tile([4, 128], f32, tag="tps")
            nc.tensor.transpose(tps2, fim, ident_t)
            nc.vector.tensor_copy(out=out_sb[:, _C + m * 128 : _C + (m + 1) * 128], in_=tps2)

        nc.gpsimd.dma_start(out=out[:], in_=out_sb)

    nc.compile()
    return nc


def _host_inputs(w1r, b1r, w1i, b1i, w2r, b2r, w2i, b2i):
    f32 = np.float32
    shared = {
        "w1rt": np.ascontiguousarray(w1r.T, dtype=f32),
        "w1it": np.ascontiguousarray(w1i.T, dtype=f32),
        "w1itn": np.ascontiguousarray(-w1i.T, dtype=f32),
        "w2rt": np.ascontiguousarray(w2r.T, dtype=f32),
        "w2it": np.ascontiguousarray(w2i.T, dtype=f32),
        "w2itn": np.ascontiguousarray(-w2i.T, dtype=f32),
        "b1re": np.ascontiguousarray((b1r - b1i).reshape(32, 1), dtype=f32),
        "b1im": np.ascontiguousarray((b1r + b1i).reshape(32, 1), dtype=f32),
        "b2re2": np.ascontiguousarray((2.0 * (b2r - b2i)).reshape(_KCH, 128), dtype=f32),
        "b2im2": np.ascontiguousarray((2.0 * (b2r + b2i)).reshape(_KCH, 128), dtype=f32),
        "ident": np.eye(128, dtype=f32),
    }
    p = np.arange(128) % 16
    dm_r = np.zeros((128, 32), dtype=f32)
    dm_r[np.arange(128), p] = 1.0
    dm_i = np.zeros((128, 32), dtype=f32)
    dm_i[np.arange(128), 16 + p] = 1.0
    shared["dmask_r"] = dm_r
    shared["dmask_i"] = dm_i
    return shared


def kernel(x, w1r, b1r, w1i, b1i, w2r, b2r, w2i, b2i):
    global last_results
    from concourse.bass_utils import run_bass_kernel_spmd

    x = np.ascontiguousarray(np.asarray(x), dtype=np.float32)
    args = [np.asarray(a, dtype=np.float32) for a in (w1r, b1r, w1i, b1i, w2r, b2r, w2i, b2i)]
    w1r, b1r, w1i, b1i, w2r, b2r, w2i, b2i = args

    if "nc" not in _STATE:
        _STATE["nc"] = _build_nc()
    nc = _STATE["nc"]

    shared = _host_inputs(w1r, b1r, w1i, b1i, w2r, b2r, w2i, b2i)
    xr3 = x.reshape(_B, _C2, _HW)
    in_maps = []
    for i in range(_NCORES):
        m = dict(shared)
        m["x"] = np.ascontiguousarray(xr3[i * _BLOC : (i + 1) * _BLOC])
        in_maps.append(m)

    trace = os.environ.get("KERNEL_TRACE", "0") == "1"
    res = run_bass_kernel_spmd(nc, in_maps, core_ids=list(range(_NCORES)), trace=trace)
    last_results = res
    return np.concatenate([r["out"] for r in res.results], axis=0)


# revision 5
# speedup vs baseline: 1.4404x; 1.1815x over previous
"""Trainium2 Bass kernel for nn_ChannelGate (pooling, complex channel attention).

Computation (per sample b):
  xr = x[b, :512], xi = x[b, 512:]            # [C, H*W]
  avg branch:  ar = mean(xr, hw), ai = mean(xi, hw)
  max branch:  score^2 = |z + 1/z|^2 = |z^2+1|^2 / |z|^2
               = ((d-1)^2 + (2 fr)^2) / d   with d = fr^2 + fi^2
               j* = argmax score^2; mr = fr[j*], mi = fi[j*]
  att = cMLP(ar, ai) + cMLP(mr, mi)           # tiny complex 2-layer MLP

Sharding: data-parallel over batch, 4 samples per core on 8 cores. The tiny
MLP weights are replicated; each core computes its own samples' outputs and
the host concatenates.

Engine budget per (b, k) tile [128 ch, 3136 hw] (the kernel is a 3-way tie
between DVE / ACT / DMA near the HBM roofline):
  DVE  3 full passes: d (+ a "spike" that deposits running-sum(fr) into a
       pad column via scan/select), nsc, and a fused score*argmax pass
       (s = nsc*y; emit Idx where s equals its running max; accum MAX
       returns the argmax directly — replaces the old mulmax+findidx pair).
  ACT  3 full passes: Ln(d), Exp(-ln) = 1/d, and Copy(fi)+accum = mean(fi).
       Activation-table thrash (Ln->natural_log, Exp->exp_and_others, 1.3us
       per swap, 2 per iter) is eliminated by restricting the table map so
       both resolve to natural_log_exp_and_others; the fixpoint pass then
       hoists the single load out of the loop.
  DMA  one 3.2MB load (the roofline term).
"""

import os

import numpy as np

_B, _C2, _H, _W = 32, 1024, 56, 56
_C = _C2 // 2
_HW = _H * _W
_HWP = _HW + 1  # spatial extent + 1 pad column for the mean-sum spike
_NCORES = 8
_BLOC = _B // _NCORES  # samples per core
_KCH = _C // 128  # channel chunks of 128

_STATE = {}
last_results = None  # BassKernelResults of the most recent run (for test.py)


def _register_ops():
    """Register the fused custom DVE ops (idempotent per process)."""
    import concourse.dve_ops as dve_ops
    from concourse.dve_spec import (
        AluOp, C0, Idx, One, Spec, Src0, Src1, Zero, eq, maxx, scan, select, sq,
    )
    from operator import add as op_add

    names = ("ANT_CG_SQ2SPK", "ANT_CG_CSCORE", "ANT_CG_ARGMAX", "ANT_CG_MULSUM")
    if names[0] in dve_ops._SUB_OPCODE_FOR_NAME:
        by_name = {op.name: op for op in dve_ops.OPS}
        return {n: by_name[n] for n in names}

    def _c_int(c):
        return int(np.asarray(c).reshape(-1)[0])

    # d = in0^2 + in1^2, except at stream position c0 where the running
    # sum of in0 (inclusive prefix) is emitted instead. With a zeroed pad
    # column at position c0 this deposits sum(fr) into d[:, c0].
    def _ref_sq2spk(in0, in1, c0, c1, c2):
        x0 = in0.astype(np.float32)
        x1 = in1.astype(np.float32)
        body = x0 * x0 + x1 * x1
        k = _c_int(c0)
        cs = np.cumsum(x0, axis=-1, dtype=np.float32)
        body[..., k] = cs[..., k]
        return body

    sq2spk_spec = Spec(
        body=select(eq(Idx, C0), scan(AluOp.ADD, Src0), sq(Src0) + sq(Src1)),
        reference=_ref_sq2spk,
    )

    # N = (in0 - 1)^2 + (c0 * in1)^2   (|z^2 + 1|^2 with in0 = |z|^2, in1 = Re z, c0 = 2)
    csc_spec = Spec(
        body=sq(Src0 - One) + sq(Src1 * C0),
        reference=lambda in0, in1, c0, c1, c2: (
            (in0.astype(np.float32) - 1.0) ** 2
            + (in1.astype(np.float32) * np.float32(c0)) ** 2
        ),
    )

    # s = in0*in1; body emits Idx where s equals its running max (prefix-max
    # positions), else 0; accum MAX of the body is the argmax of s (last
    # occurrence on exact float ties — measure-zero for this input).
    def _ref_argmax(in0, in1, c0, c1, c2):
        s = in0.astype(np.float32) * in1.astype(np.float32)
        m = np.maximum.accumulate(s, axis=-1)
        idxs = np.arange(s.shape[-1], dtype=np.float32)
        body = np.where(s == m, idxs, 0.0).astype(np.float32)
        return body, body.max(axis=-1, keepdims=True)

    _s = Src0 * Src1
    argmax_spec = Spec(
        body=select(eq(_s, scan(AluOp.MAX, _s)), Idx, Zero),
        accum=maxx,
        reference=_ref_argmax,
    )

    def _mul(in0, in1):
        return in0.astype(np.float32) * in1

    # out = in0*in1; accum = sum(out)
    mulsum_spec = Spec(
        body=Src0 * Src1,
        accum=op_add,
        reference=lambda in0, in1, c0, c1, c2: (
            _mul(in0, in1),
            _mul(in0, in1).reshape(in0.shape[0], -1).sum(axis=-1, keepdims=True),
        ),
    )

    ops = {}
    for name, spec in zip(
        names, (sq2spk_spec, csc_spec, argmax_spec, mulsum_spec)
    ):
        op = dve_ops.DveOp(name, spec, subdim=False, uops_sha={})
        dve_ops.OPS.append(op)
        dve_ops.CUSTOM_DVE_SPECS[name] = spec
        dve_ops._SUB_OPCODE_FOR_NAME[name] = (
            max(dve_ops._SUB_OPCODE_FOR_NAME.values()) + 1
        )
        for ver in ("v3", "v4"):
            try:
                sha = dve_ops.DveOpSpec(
                    name=name,
                    opcode=dve_ops.get_dve_sub_opcode(name),
                    uops=dve_ops.lower(spec, ver=ver),
                    rd1_en=dve_ops.has_src1(spec),
                ).sha(ver)
                op.uops_sha[ver] = sha
            except Exception:
                pass
        ops[name] = op
    return ops


def _patch_act_tables():
    """Pin Ln and Exp to the one table set containing both.

    The table-load placement pass assigns each activation the FIRST set
    containing its function (Ln -> natural_log, Exp -> exp_and_others),
    which costs two 1.3us ACT_TABLE_LOADs per loop iteration. Removing
    ln/exp from every other set (indices untouched) makes both resolve to
    natural_log_exp_and_others, and the fixpoint hoists the load out of
    the loop entirely.
    """
    import concourse.bacc as bacc_mod
    from concourse import mybir

    AF = mybir.ActivationFunctionType
    orig = bacc_mod.get_activation_tables
    if getattr(orig, "_ant_cg_patched", False):
        return
    def patched(arch):
        t = {}
        for name, funcs in orig(arch).items():
            funcs = set(funcs)
            if name != "natural_log_exp_and_others":
                funcs.discard(AF.Ln)
                funcs.discard(AF.Exp)
            t[name] = funcs
        return t
    patched._ant_cg_patched = True
    bacc_mod.get_activation_tables = patched


def _build_nc(repeat=1):
    ops = _register_ops()
    _patch_act_tables()
    from contextlib import ExitStack

    import concourse.bacc as bacc
    import concourse.tile as tile
    from concourse import mybir

    f32 = mybir.dt.float32
    u16 = mybir.dt.uint16
    A = mybir.AluOpType
    AF = mybir.ActivationFunctionType
    SQ2SPK = ops["ANT_CG_SQ2SPK"]
    CSC = ops["ANT_CG_CSCORE"]
    ARGMAX = ops["ANT_CG_ARGMAX"]
    MULSUM = ops["ANT_CG_MULSUM"]

    nc = bacc.Bacc("TRN2", target_bir_lowering=False, debug=False)
    x = nc.dram_tensor("x", [_BLOC, _C2, _HW], f32, kind="ExternalInput")
    w1rt = nc.dram_tensor("w1rt", [_C, 32], f32, kind="ExternalInput")
    w1it = nc.dram_tensor("w1it", [_C, 32], f32, kind="ExternalInput")
    w1itn = nc.dram_tensor("w1itn", [_C, 32], f32, kind="ExternalInput")
    w2rt = nc.dram_tensor("w2rt", [32, _C], f32, kind="ExternalInput")
    w2it = nc.dram_tensor("w2it", [32, _C], f32, kind="ExternalInput")
    w2itn = nc.dram_tensor("w2itn", [32, _C], f32, kind="ExternalInput")
    b1re = nc.dram_tensor("b1re", [32, 1], f32, kind="ExternalInput")
    b1im = nc.dram_tensor("b1im", [32, 1], f32, kind="ExternalInput")
    b2re2 = nc.dram_tensor("b2re2", [_KCH, 128], f32, kind="ExternalInput")
    b2im2 = nc.dram_tensor("b2im2", [_KCH, 128], f32, kind="ExternalInput")
    ident = nc.dram_tensor("ident", [128, 128], f32, kind="ExternalInput")
    dmask_r = nc.dram_tensor("dmask_r", [128, 32], f32, kind="ExternalInput")
    dmask_i = nc.dram_tensor("dmask_i", [128, 32], f32, kind="ExternalInput")
    out = nc.dram_tensor("out", [_BLOC, _C2], f32, kind="ExternalOutput")

    with ExitStack() as ctx:
        tc = ctx.enter_context(tile.TileContext(nc))
        singles = ctx.enter_context(tc.tile_pool(name="singles", bufs=1))
        work = ctx.enter_context(tc.tile_pool(name="work", bufs=2))
        workx = ctx.enter_context(tc.tile_pool(name="workx", bufs=3))
        small = ctx.enter_context(tc.tile_pool(name="small", bufs=2))
        mlp = ctx.enter_context(tc.tile_pool(name="mlp", bufs=1))
        psum = ctx.enter_context(tc.tile_pool(name="psum", bufs=2, space="PSUM"))

        # --- constants ---
        w1rt_t = singles.tile([128, _KCH, 32], f32)
        nc.gpsimd.dma_start(out=w1rt_t, in_=w1rt[:].rearrange("(k p) j -> p k j", p=128))
        w1it_t = singles.tile([128, _KCH, 32], f32)
        nc.gpsimd.dma_start(out=w1it_t, in_=w1it[:].rearrange("(k p) j -> p k j", p=128))
        w1itn_t = singles.tile([128, _KCH, 32], f32)
        nc.gpsimd.dma_start(
            out=w1itn_t, in_=w1itn[:].rearrange("(k p) j -> p k j", p=128)
        )
        w2rt_t = singles.tile([32, _C], f32)
        nc.gpsimd.dma_start(out=w2rt_t, in_=w2rt[:])
        w2it_t = singles.tile([32, _C], f32)
        nc.gpsimd.dma_start(out=w2it_t, in_=w2it[:])
        w2itn_t = singles.tile([32, _C], f32)
        nc.gpsimd.dma_start(out=w2itn_t, in_=w2itn[:])
        b1re_t = singles.tile([32, 1], f32)
        nc.gpsimd.dma_start(out=b1re_t, in_=b1re[:])
        b1im_t = singles.tile([32, 1], f32)
        nc.gpsimd.dma_start(out=b1im_t, in_=b1im[:])
        b2re2_t = singles.tile([128, _KCH], f32)
        nc.gpsimd.dma_start(out=b2re2_t, in_=b2re2[:].rearrange("k p -> p k"))
        b2im2_t = singles.tile([128, _KCH], f32)
        nc.gpsimd.dma_start(out=b2im2_t, in_=b2im2[:].rearrange("k p -> p k"))
        ident_t = singles.tile([128, 128], f32)
        nc.gpsimd.dma_start(out=ident_t, in_=ident[:])
        dmask_r_t = singles.tile([128, 32], f32)
        nc.gpsimd.dma_start(out=dmask_r_t, in_=dmask_r[:])
        dmask_i_t = singles.tile([128, 32], f32)
        nc.gpsimd.dma_start(out=dmask_i_t, in_=dmask_i[:])

        junk32 = singles.tile([128, 32], f32)
        # MLP inputs, transposed: [channel, sample-column]; cols 0-3 avg, 4-7 max
        stage_re = singles.tile([128, _KCH, 8], f32)
        stage_im = singles.tile([128, _KCH, 8], f32)
        # means staging, merged into stage_* before the MLP so the matmuls
        # depend on a single writer engine.
        stage_avg_re = singles.tile([128, _KCH, 4], f32)
        stage_avg_im = singles.tile([128, _KCH, 4], f32)
        # Touch the mask constants on DVE once so the per-iteration ISA-encoded
        # DVE ops (1 wait slot only) never wait on these DMAs directly.
        nc.vector.tensor_copy(out=junk32, in_=dmask_r_t)
        nc.vector.tensor_copy(out=junk32, in_=dmask_i_t)

        xv = x[:]

        iters = [(b, k) for _ in range(repeat)
                 for b in range(_BLOC) for k in range(_KCH)]
        n_iter = len(iters)
        xtiles = {}

        def issue_load(j):
            # X loads are issued PREFETCH iterations ahead of consumption so
            # the in-order DVE stream never head-of-line blocks on a transfer.
            b, k = iters[j]
            X = workx.tile([128, 2, _HWP], f32, tag="X")
            # zero the pad column (both halves) so the fr prefix-sum spike is
            # exact and the pad never wins anything
            nc.gpsimd.memset(X[:, :, _HW:_HWP], 0.0)
            # one DMA for both halves (real chunk k, imag chunk k) on SP HWDGE
            src = xv[b].rearrange("(j c) w -> c j w", j=2)[k * 128 : (k + 1) * 128]
            nc.sync.dma_start(out=X[:, :, 0:_HW], in_=src)
            xtiles[j] = X

        # Software pipeline: stage A (iter i): d/spike + 1/d + N + means.
        # Stage B (emitted during iter i+1): fused argmax, gather.
        # Stage C (emitted during iter i+2): masked-reduce extraction.
        def emit_stage_b(st):
            # acc = argmax_j (nsc[j] * y[j]) as f32, single fused pass; the
            # body output lands in the dead d tile of the same iteration
            acc = small.tile([128, 1], f32, tag="acc")
            nc.vector._custom_dve(
                ARGMAX, out=st["d"][:, 0:_HW], in0=st["nsc"], in1=st["y"],
                accum_out=acc,
            )
            # gather winners: per 16-partition group, fetch all 16 indices;
            # the (p, p%16) diagonal is extracted in stage C.
            # idx2 = [j, HWP + j] as uint16 (fi half starts at offset HWP)
            idx2 = small.tile([128, 2], u16, tag="idx2")
            nc.gpsimd.tensor_scalar(
                out=idx2[:, 0:1], in0=acc, scalar1=1.0, scalar2=0.0,
                op0=A.mult, op1=A.add,
            )
            nc.gpsimd.tensor_scalar(
                out=idx2[:, 1:2], in0=acc, scalar1=1.0, scalar2=float(_HWP),
                op0=A.mult, op1=A.add,
            )
            gath = small.tile([128, 32], f32, tag="gath")
            nc.gpsimd.indirect_copy(
                out=gath, data=st["X"][:].rearrange("p a b -> p (a b)"), idxs=idx2,
                i_know_ap_gather_is_preferred=True,
            )
            return {"gath": gath, "k": st["k"], "b": st["b"]}

        def emit_stage_c(st):
            nc.vector._custom_dve(
                MULSUM, out=junk32, in0=st["gath"], in1=dmask_r_t,
                accum_out=stage_re[:, st["k"], 4 + st["b"] : 5 + st["b"]],
            )
            nc.vector._custom_dve(
                MULSUM, out=junk32, in0=st["gath"], in1=dmask_i_t,
                accum_out=stage_im[:, st["k"], 4 + st["b"] : 5 + st["b"]],
            )

        _PREFETCH = 2  # X pool bufs = _PREFETCH + 1
        for j in range(min(_PREFETCH + 1, n_iter)):
            issue_load(j)

        prev1 = None
        prev2 = None
        for j, (b, k) in enumerate(iters):
                X = xtiles.pop(j)
                fr = X[:, 0, :]
                fi = X[:, 1, :]

                # d = fr^2 + fi^2 over [0:HW); d[HW] = sum(fr) via the spike
                d = work.tile([128, _HWP], f32, tag="d")
                nc.vector._custom_dve(
                    SQ2SPK, out=d, in0=fr, in1=fi, s0=float(_HW)
                )
                # mean(fi) on ACT; the throwaway elementwise output goes into
                # the y tile, which Ln overwrites right after (same engine)
                y = work.tile([128, _HW], f32, tag="y")
                nc.scalar.activation(
                    out=y, in_=X[:, 1, 0:_HW], func=AF.Copy, bias=0.0,
                    scale=1.0 / _HW,
                    accum_out=stage_avg_im[:, k, b : b + 1],
                )
                # y = 1/d on ACT via exp(-ln d); both live in one table set
                nc.scalar.activation(out=y, in_=d[:, 0:_HW], func=AF.Ln)
                nc.scalar.activation(out=y, in_=y, func=AF.Exp, scale=-1.0)

                # stage B of the previous iteration: its Exp dependency has
                # had a full iteration to complete, so this never stalls DVE
                nxt2 = emit_stage_b(prev1) if prev1 is not None else None

                nsc = work.tile([128, _HW], f32, tag="nsc")
                nc.vector._custom_dve(
                    CSC, out=nsc, in0=d[:, 0:_HW], in1=X[:, 0, 0:_HW], s0=2.0
                )
                # mean(fr) = spike / HW
                nc.gpsimd.tensor_scalar(
                    out=stage_avg_re[:, k, b : b + 1], in0=d[:, _HW:_HWP],
                    scalar1=1.0 / _HW, scalar2=0.0, op0=A.mult, op1=A.add,
                )

                if prev2 is not None:
                    emit_stage_c(prev2)
                prev2 = nxt2
                prev1 = {"nsc": nsc, "y": y, "X": X, "d": d, "k": k, "b": b}
                if j + _PREFETCH + 1 < n_iter:
                    issue_load(j + _PREFETCH + 1)
        # drain the pipeline
        nxt2 = emit_stage_b(prev1)
        if prev2 is not None:
            emit_stage_c(prev2)
        if nxt2 is not None:
            emit_stage_c(nxt2)

        # --- tiny complex MLP on PE (transposed layout [feature, column]) ---
        nc.vector.tensor_copy(out=stage_re[:, :, 0:4], in_=stage_avg_re)
        nc.vector.tensor_copy(out=stage_im[:, :, 0:4], in_=stage_avg_im)
        hps = psum.tile([32, 2, 8], f32, tag="hps")
        for k in range(_KCH):
            nc.tensor.matmul(
                hps[:, 0, :], lhsT=w1rt_t[:, k, :], rhs=stage_re[:, k, :],
                start=(k == 0), stop=False,
            )
        for k in range(_KCH):
            nc.tensor.matmul(
                hps[:, 0, :], lhsT=w1itn_t[:, k, :], rhs=stage_im[:, k, :],
                start=False, stop=(k == _KCH - 1),
            )
        for k in range(_KCH):
            nc.tensor.matmul(
                hps[:, 1, :], lhsT=w1rt_t[:, k, :], rhs=stage_im[:, k, :],
                start=(k == 0), stop=False,
            )
        for k in range(_KCH):
            nc.tensor.matmul(
                hps[:, 1, :], lhsT=w1it_t[:, k, :], rhs=stage_re[:, k, :],
                start=False, stop=(k == _KCH - 1),
            )
        hreT = mlp.tile([32, 8], f32)
        nc.vector.tensor_scalar(
            out=hreT, in0=hps[:, 0, :], scalar1=b1re_t, scalar2=None, op0=A.add
        )
        himT = mlp.tile([32, 8], f32)
        nc.vector.tensor_scalar(
            out=himT, in0=hps[:, 1, :], scalar1=b1im_t, scalar2=None, op0=A.add
        )

        # cardioid: s = 0.5 * (1 + re / |h|)
        q2 = mlp.tile([32, 8], f32)
        nc.vector.tensor_tensor(out=q2, in0=hreT, in1=hreT, op=A.mult)
        q2b = mlp.tile([32, 8], f32)
        nc.vector.tensor_tensor(out=q2b, in0=himT, in1=himT, op=A.mult)
        nc.vector.tensor_tensor(out=q2, in0=q2, in1=q2b, op=A.add)
        ah = mlp.tile([32, 8], f32)
        nc.scalar.activation(out=ah, in_=q2, func=AF.Sqrt)
        rh = mlp.tile([32, 8], f32)
        nc.vector.reciprocal(out=rh, in_=ah)
        s = mlp.tile([32, 8], f32)
        nc.vector.tensor_tensor(out=s, in0=hreT, in1=rh, op=A.mult)
        nc.vector.tensor_scalar(out=s, in0=s, scalar1=0.5, scalar2=0.5, op0=A.mult, op1=A.add)
        greT = mlp.tile([32, 8], f32)
        nc.vector.tensor_tensor(out=greT, in0=hreT, in1=s, op=A.mult)
        gimT = mlp.tile([32, 8], f32)
        nc.vector.tensor_tensor(out=gimT, in0=himT, in1=s, op=A.mult)

        out_sb = singles.tile([_BLOC, _C2], f32)
        for m in range(_KCH):
            sl = slice(m * 128, (m + 1) * 128)
            ore = psum.tile([128, 8], f32, tag="ore")
            nc.tensor.matmul(ore, lhsT=w2rt_t[:, sl], rhs=greT, start=True, stop=False)
            nc.tensor.matmul(ore, lhsT=w2itn_t[:, sl], rhs=gimT, start=False, stop=True)
            osb_re = mlp.tile([128, 8], f32, tag="osb")
            nc.scalar.copy(out=osb_re, in_=ore)
            fre = mlp.tile([128, 4], f32, tag="fre")
            nc.vector.tensor_tensor(out=fre, in0=osb_re[:, 0:4], in1=osb_re[:, 4:8], op=A.add)
            nc.vector.tensor_scalar(
                out=fre, in0=fre, scalar1=b2re2_t[:, m : m + 1], scalar2=None, op0=A.add
            )
            tps = psum.tile([4, 128], f32, tag="tps")
            nc.tensor.transpose(tps, fre, ident_t)
            nc.vector.tensor_copy(out=out_sb[:, sl], in_=tps)

            oim = psum.tile([128, 8], f32, tag="oim")
            nc.tensor.matmul(oim, lhsT=w2it_t[:, sl], rhs=greT, start=True, stop=False)
            nc.tensor.matmul(oim, lhsT=w2rt_t[:, sl], rhs=gimT, start=False, stop=True)
            osb_im = mlp.tile([128, 8], f32, tag="osb")
            nc.scalar.copy(out=osb_im, in_=oim)
            fim = mlp.tile([128, 4], f32, tag="fim")
            nc.vector.tensor_tensor(out=fim, in0=osb_im[:, 0:4], in1=osb_im[:, 4:8], op=A.add)
            nc.vector.tensor_scalar(
                out=fim, in0=fim, scalar1=b2im2_t[:, m : m + 1], scalar2=None, op0=A.add
            )
            tps2 = psum.tile([4, 128], f32, tag="tps")
            nc.tensor.transpose(tps2, fim, ident_t)
            nc.vector.tensor_copy(out=out_sb[:, _C + m * 128 : _C + (m + 1) * 128], in_=tps2)

        nc.gpsimd.dma_start(out=out[:], in_=out_sb)

    nc.compile()
    return nc


def _host_inputs(w1r, b1r, w1i, b1i, w2r, b2r, w2i, b2i):
    f32 = np.float32
    shared = {
        "w1rt": np.ascontiguousarray(w1r.T, dtype=f32),
        "w1it": np.ascontiguousarray(w1i.T, dtype=f32),
        "w1itn": np.ascontiguousarray(-w1i.T, dtype=f32),
        "w2rt": np.ascontiguousarray(w2r.T, dtype=f32),
        "w2it": np.ascontiguousarray(w2i.T, dtype=f32),
        "w2itn": np.ascontiguousarray(-w2i.T, dtype=f32),
        "b1re": np.ascontiguousarray((b1r - b1i).reshape(32, 1), dtype=f32),
        "b1im": np.ascontiguousarray((b1r + b1i).reshape(32, 1), dtype=f32),
        "b2re2": np.ascontiguousarray((2.0 * (b2r - b2i)).reshape(_KCH, 128), dtype=f32),
        "b2im2": np.ascontiguousarray((2.0 * (b2r + b2i)).reshape(_KCH, 128), dtype=f32),
        "ident": np.eye(128, dtype=f32),
    }
    p = np.arange(128) % 16
    dm_r = np.zeros((128, 32), dtype=f32)
    dm_r[np.arange(128), p] = 1.0
    dm_i = np.zeros((128, 32), dtype=f32)
    dm_i[np.arange(128), 16 + p] = 1.0
    shared["dmask_r"] = dm_r
    shared["dmask_i"] = dm_i
    return shared


def kernel(x, w1r, b1r, w1i, b1i, w2r, b2r, w2i, b2i):
    global last_results
    from concourse.bass_utils import run_bass_kernel_spmd

    x = np.ascontiguousarray(np.asarray(x), dtype=np.float32)
    args = [np.asarray(a, dtype=np.float32) for a in (w1r, b1r, w1i, b1i, w2r, b2r, w2i, b2i)]
    w1r, b1r, w1i, b1i, w2r, b2r, w2i, b2i = args

    if "nc" not in _STATE:
        _STATE["nc"] = _build_nc()
    nc = _STATE["nc"]

    shared = _host_inputs(w1r, b1r, w1i, b1i, w2r, b2r, w2i, b2i)
    xr3 = x.reshape(_B, _C2, _HW)
    in_maps = []
    for i in range(_NCORES):
        m = dict(shared)
        m["x"] = np.ascontiguousarray(xr3[i * _BLOC : (i + 1) * _BLOC])
        in_maps.append(m)

    trace = os.environ.get("KERNEL_TRACE", "0") == "1"
    res = run_bass_kernel_spmd(nc, in_maps, core_ids=list(range(_NCORES)), trace=trace)
    last_results = res
    return np.concatenate([r["out"] for r in res.results], axis=0)


# revision 20
# speedup vs baseline: 1.5746x; 1.0932x over previous
"""Trainium2 Bass kernel for nn_ChannelGate (pooling, complex channel attention).

Computation (per sample b):
  xr = x[b, :512], xi = x[b, 512:]            # [C, H*W]
  avg branch:  ar = mean(xr, hw), ai = mean(xi, hw)
  max branch:  score^2 = |z + 1/z|^2 = |z^2+1|^2 / |z|^2
               = ((d-1)^2 + (2 fr)^2) / d   with d = fr^2 + fi^2
               j* = argmax score^2; mr = fr[j*], mi = fi[j*]
  att = cMLP(ar, ai) + cMLP(mr, mi)           # tiny complex 2-layer MLP

Sharding: data-parallel over batch, 4 samples per core on 8 cores. The tiny
MLP weights are replicated; each core computes its own samples' outputs and
the host concatenates.

Engine budget per (b, k) tile [128 ch, 3136 hw] (the kernel is a 3-way tie
between DVE / ACT / DMA near the HBM roofline):
  DVE  3 full passes: d (+ a "spike" that deposits running-sum(fr) into a
       pad column via scan/select), nsc, and a fused score*argmax pass
       (s = nsc*y; emit Idx where s equals its running max; accum MAX
       returns the argmax directly — replaces the old mulmax+findidx pair).
  ACT  3 full passes: Ln(d), Exp(-ln) = 1/d, and Copy(fi)+accum = mean(fi).
       Activation-table thrash (Ln->natural_log, Exp->exp_and_others, 1.3us
       per swap, 2 per iter) is eliminated by restricting the table map so
       both resolve to natural_log_exp_and_others; the fixpoint pass then
       hoists the single load out of the loop.
  DMA  one 3.2MB load (the roofline term).
"""

import os

import numpy as np

_B, _C2, _H, _W = 32, 1024, 56, 56
_C = _C2 // 2
_HW = _H * _W
_HWP = _HW + 1  # spatial extent + 1 pad column for the mean-sum spike
_NCORES = 8
_BLOC = _B // _NCORES  # samples per core
_KCH = _C // 128  # channel chunks of 128

_STATE = {}
last_results = None  # BassKernelResults of the most recent run (for test.py)


def _register_ops():
    """Register the fused custom DVE ops (idempotent per process)."""
    import concourse.dve_ops as dve_ops
    from concourse.dve_spec import (
        AluOp, C0, Idx, One, Spec, Src0, Src1, Zero, eq, maxx, scan, select, sq,
    )
    from operator import add as op_add

    names = ("ANT_CG_SQ2SPK", "ANT_CG_CSCORE", "ANT_CG_ARGMAX", "ANT_CG_MULSUM")
    if names[0] in dve_ops._SUB_OPCODE_FOR_NAME:
        by_name = {op.name: op for op in dve_ops.OPS}
        return {n: by_name[n] for n in names}

    def _c_int(c):
        return int(np.asarray(c).reshape(-1)[0])

    # d = in0^2 + in1^2, except at stream position c0 where the running
    # sum of in0 (inclusive prefix) is emitted instead. With a zeroed pad
    # column at position c0 this deposits sum(fr) into d[:, c0].
    def _ref_sq2spk(in0, in1, c0, c1, c2):
        x0 = in0.astype(np.float32)
        x1 = in1.astype(np.float32)
        body = x0 * x0 + x1 * x1
        k = _c_int(c0)
        if k < body.shape[-1]:
            cs = np.cumsum(x0, axis=-1, dtype=np.float32)
            body[..., k] = cs[..., k]
        return body

    sq2spk_spec = Spec(
        body=select(eq(Idx, C0), scan(AluOp.ADD, Src0), sq(Src0) + sq(Src1)),
        reference=_ref_sq2spk,
    )

    # N = (in0 - 1)^2 + (c0 * in1)^2   (|z^2 + 1|^2 with in0 = |z|^2, in1 = Re z, c0 = 2)
    csc_spec = Spec(
        body=sq(Src0 - One) + sq(Src1 * C0),
        reference=lambda in0, in1, c0, c1, c2: (
            (in0.astype(np.float32) - 1.0) ** 2
            + (in1.astype(np.float32) * np.float32(c0)) ** 2
        ),
    )

    # s = in0*in1; body emits Idx where s equals its running max (prefix-max
    # positions), else 0; accum MAX of the body is the argmax of s (last
    # occurrence on exact float ties — measure-zero for this input).
    def _ref_argmax(in0, in1, c0, c1, c2):
        s = in0.astype(np.float32) * in1.astype(np.float32)
        m = np.maximum.accumulate(s, axis=-1)
        idxs = np.arange(s.shape[-1], dtype=np.float32)
        body = np.where(s == m, idxs, 0.0).astype(np.float32)
        return body, body.max(axis=-1, keepdims=True)

    _s = Src0 * Src1
    argmax_spec = Spec(
        body=select(eq(_s, scan(AluOp.MAX, _s)), Idx, Zero),
        accum=maxx,
        reference=_ref_argmax,
    )

    def _mul(in0, in1):
        return in0.astype(np.float32) * in1

    # out = in0*in1; accum = sum(out)
    mulsum_spec = Spec(
        body=Src0 * Src1,
        accum=op_add,
        reference=lambda in0, in1, c0, c1, c2: (
            _mul(in0, in1),
            _mul(in0, in1).reshape(in0.shape[0], -1).sum(axis=-1, keepdims=True),
        ),
    )

    ops = {}
    for name, spec in zip(
        names, (sq2spk_spec, csc_spec, argmax_spec, mulsum_spec)
    ):
        op = dve_ops.DveOp(name, spec, subdim=False, uops_sha={})
        dve_ops.OPS.append(op)
        dve_ops.CUSTOM_DVE_SPECS[name] = spec
        dve_ops._SUB_OPCODE_FOR_NAME[name] = (
            max(dve_ops._SUB_OPCODE_FOR_NAME.values()) + 1
        )
        for ver in ("v3", "v4"):
            try:
                sha = dve_ops.DveOpSpec(
                    name=name,
                    opcode=dve_ops.get_dve_sub_opcode(name),
                    uops=dve_ops.lower(spec, ver=ver),
                    rd1_en=dve_ops.has_src1(spec),
                ).sha(ver)
                op.uops_sha[ver] = sha
            except Exception:
                pass
        ops[name] = op
    return ops


def _patch_act_tables():
    """Pin Ln and Exp to the one table set containing both.

    The table-load placement pass assigns each activation the FIRST set
    containing its function (Ln -> natural_log, Exp -> exp_and_others),
    which costs two 1.3us ACT_TABLE_LOADs per loop iteration. Removing
    ln/exp from every other set (indices untouched) makes both resolve to
    natural_log_exp_and_others, and the fixpoint hoists the load out of
    the loop entirely.
    """
    import concourse.bacc as bacc_mod
    from concourse import mybir

    AF = mybir.ActivationFunctionType
    orig = bacc_mod.get_activation_tables
    if getattr(orig, "_ant_cg_patched", False):
        return
    def patched(arch):
        t = {}
        for name, funcs in orig(arch).items():
            funcs = set(funcs)
            if name != "natural_log_exp_and_others":
                funcs.discard(AF.Ln)
                funcs.discard(AF.Exp)
            t[name] = funcs
        return t
    patched._ant_cg_patched = True
    bacc_mod.get_activation_tables = patched


def _build_nc(repeat=1):
    ops = _register_ops()
    _patch_act_tables()
    from contextlib import ExitStack

    import concourse.bacc as bacc
    import concourse.tile as tile
    from concourse import mybir

    f32 = mybir.dt.float32
    u16 = mybir.dt.uint16
    A = mybir.AluOpType
    AF = mybir.ActivationFunctionType
    SQ2SPK = ops["ANT_CG_SQ2SPK"]
    CSC = ops["ANT_CG_CSCORE"]
    ARGMAX = ops["ANT_CG_ARGMAX"]
    MULSUM = ops["ANT_CG_MULSUM"]

    nc = bacc.Bacc("TRN2", target_bir_lowering=False, debug=False)
    x = nc.dram_tensor("x", [_BLOC, _C2, _HW], f32, kind="ExternalInput")
    w1rt = nc.dram_tensor("w1rt", [_C, 32], f32, kind="ExternalInput")
    w1it = nc.dram_tensor("w1it", [_C, 32], f32, kind="ExternalInput")
    w1itn = nc.dram_tensor("w1itn", [_C, 32], f32, kind="ExternalInput")
    w2rt = nc.dram_tensor("w2rt", [32, _C], f32, kind="ExternalInput")
    w2it = nc.dram_tensor("w2it", [32, _C], f32, kind="ExternalInput")
    w2itn = nc.dram_tensor("w2itn", [32, _C], f32, kind="ExternalInput")
    b1re = nc.dram_tensor("b1re", [32, 1], f32, kind="ExternalInput")
    b1im = nc.dram_tensor("b1im", [32, 1], f32, kind="ExternalInput")
    b2re2 = nc.dram_tensor("b2re2", [_KCH, 128], f32, kind="ExternalInput")
    b2im2 = nc.dram_tensor("b2im2", [_KCH, 128], f32, kind="ExternalInput")
    ident = nc.dram_tensor("ident", [128, 128], f32, kind="ExternalInput")
    dmask_r = nc.dram_tensor("dmask_r", [128, 32], f32, kind="ExternalInput")
    dmask_i = nc.dram_tensor("dmask_i", [128, 32], f32, kind="ExternalInput")
    out = nc.dram_tensor("out", [_BLOC, _C2], f32, kind="ExternalOutput")

    with ExitStack() as ctx:
        tc = ctx.enter_context(tile.TileContext(nc))
        singles = ctx.enter_context(tc.tile_pool(name="singles", bufs=1))
        work = ctx.enter_context(tc.tile_pool(name="work", bufs=2))
        workx = ctx.enter_context(tc.tile_pool(name="workx", bufs=4))
        small = ctx.enter_context(tc.tile_pool(name="small", bufs=2))
        mlp = ctx.enter_context(tc.tile_pool(name="mlp", bufs=1))
        psum = ctx.enter_context(tc.tile_pool(name="psum", bufs=2, space="PSUM"))
        psum1 = ctx.enter_context(tc.tile_pool(name="psum1", bufs=1, space="PSUM"))

        # --- constants ---
        w1rt_t = singles.tile([128, _KCH, 32], f32)
        nc.gpsimd.dma_start(out=w1rt_t, in_=w1rt[:].rearrange("(k p) j -> p k j", p=128))
        w1it_t = singles.tile([128, _KCH, 32], f32)
        nc.gpsimd.dma_start(out=w1it_t, in_=w1it[:].rearrange("(k p) j -> p k j", p=128))
        w1itn_t = singles.tile([128, _KCH, 32], f32)
        nc.gpsimd.dma_start(
            out=w1itn_t, in_=w1itn[:].rearrange("(k p) j -> p k j", p=128)
        )
        w2rt_t = singles.tile([32, _C], f32)
        nc.gpsimd.dma_start(out=w2rt_t, in_=w2rt[:])
        w2it_t = singles.tile([32, _C], f32)
        nc.gpsimd.dma_start(out=w2it_t, in_=w2it[:])
        w2itn_t = singles.tile([32, _C], f32)
        nc.gpsimd.dma_start(out=w2itn_t, in_=w2itn[:])
        b1re_t = singles.tile([32, 1], f32)
        nc.gpsimd.dma_start(out=b1re_t, in_=b1re[:])
        b1im_t = singles.tile([32, 1], f32)
        nc.gpsimd.dma_start(out=b1im_t, in_=b1im[:])
        b2re2_t = singles.tile([128, _KCH], f32)
        nc.gpsimd.dma_start(out=b2re2_t, in_=b2re2[:].rearrange("k p -> p k"))
        b2im2_t = singles.tile([128, _KCH], f32)
        nc.gpsimd.dma_start(out=b2im2_t, in_=b2im2[:].rearrange("k p -> p k"))
        ident_t = singles.tile([128, 128], f32)
        nc.gpsimd.dma_start(out=ident_t, in_=ident[:])
        dmask_r_t = singles.tile([128, 32], f32)
        nc.gpsimd.dma_start(out=dmask_r_t, in_=dmask_r[:])
        dmask_i_t = singles.tile([128, 32], f32)
        nc.gpsimd.dma_start(out=dmask_i_t, in_=dmask_i[:])

        junk32 = singles.tile([128, 32], f32)
        # MLP inputs, transposed: [channel, sample-column]; cols 0-3 avg, 4-7 max
        stage_re = singles.tile([128, _KCH, 8], f32)
        stage_im = singles.tile([128, _KCH, 8], f32)

        xv = x[:]

        # k-major order: channel chunk k is fully staged after its 4 samples
        # drain, so its first-layer matmuls can interleave with the loop
        iters = [(b, k) for _ in range(repeat)
                 for k in range(_KCH) for b in range(_BLOC)]
        n_iter = len(iters)
        xtiles = {}

        def issue_load(j):
            # X loads are issued PREFETCH iterations ahead of consumption so
            # the in-order DVE stream never head-of-line blocks on a transfer.
            b, k = iters[j]
            X = workx.tile([128, 2, _HWP], f32, tag="X")
            # zero the pad column (both halves) so the fr prefix-sum spike is
            # exact and the pad never wins anything
            nc.gpsimd.memset(X[:, :, _HW:_HWP], 0.0)
            # one DMA for both halves (real chunk k, imag chunk k) on SP HWDGE
            src = xv[b].rearrange("(j c) w -> c j w", j=2)[k * 128 : (k + 1) * 128]
            nc.sync.dma_start(out=X[:, :, 0:_HW], in_=src)
            xtiles[j] = X

        # Software pipeline: stage A (iter i): d/spike + 1/d + N + means.
        # Stage B (emitted during iter i+1): fused argmax, gather.
        # Stage C (emitted during iter i+2): masked-reduce extraction.
        def emit_stage_b(st):
            # acc = argmax_j (nsc[j] * y[j]) as f32, single fused pass; the
            # body output lands in the dead d tile of the same iteration
            acc = small.tile([128, 1], f32, tag="acc")
            nc.vector._custom_dve(
                ARGMAX, out=st["d"][:, 0:_HW], in0=st["nsc"], in1=st["y"],
                accum_out=acc,
            )
            # gather winners: per 16-partition group, fetch all 16 indices;
            # the (p, p%16) diagonal is extracted in stage C.
            # idx2 = [j, HWP + j] as uint16 (fi half starts at offset HWP)
            idx2 = small.tile([128, 2], u16, tag="idx2")
            nc.gpsimd.tensor_scalar(
                out=idx2[:, 0:1], in0=acc, scalar1=1.0, scalar2=0.0,
                op0=A.mult, op1=A.add,
            )
            nc.gpsimd.tensor_scalar(
                out=idx2[:, 1:2], in0=acc, scalar1=1.0, scalar2=float(_HWP),
                op0=A.mult, op1=A.add,
            )
            gath = small.tile([128, 32], f32, tag="gath")
            nc.gpsimd.indirect_copy(
                out=gath, data=st["X"][:].rearrange("p a b -> p (a b)"), idxs=idx2,
                i_know_ap_gather_is_preferred=True,
            )
            return {"gath": gath, "k": st["k"], "b": st["b"]}

        def emit_stage_c(st):
            nc.vector._custom_dve(
                MULSUM, out=junk32, in0=st["gath"], in1=dmask_r_t,
                accum_out=stage_re[:, st["k"], 4 + st["b"] : 5 + st["b"]],
            )
            nc.vector._custom_dve(
                MULSUM, out=junk32, in0=st["gath"], in1=dmask_i_t,
                accum_out=stage_im[:, st["k"], 4 + st["b"] : 5 + st["b"]],
            )
            if st["b"] == _BLOC - 1 and repeat == 1:
                emit_l1_matmuls(st["k"])

        # first MLP layer, one accumulation group per channel chunk, emitted
        # as soon as that chunk's stage columns are complete. The re/im
        # accumulation chains stay pending simultaneously across the loop, so
        # each needs its OWN psum zero region (2KB bank): starting a second
        # group in a pending group's region corrupts it (sim enforces this).
        hps0 = psum1.tile([32, 512], f32, tag="hps0")
        hps1 = psum1.tile([32, 512], f32, tag="hps1")

        def emit_l1_matmuls(k):
            nc.tensor.matmul(
                hps0[:, 0:8], lhsT=w1rt_t[:, k, :], rhs=stage_re[:, k, :],
                start=(k == 0), stop=False,
            )
            nc.tensor.matmul(
                hps0[:, 0:8], lhsT=w1itn_t[:, k, :], rhs=stage_im[:, k, :],
                start=False, stop=(k == _KCH - 1),
            )
            nc.tensor.matmul(
                hps1[:, 0:8], lhsT=w1rt_t[:, k, :], rhs=stage_im[:, k, :],
                start=(k == 0), stop=False,
            )
            nc.tensor.matmul(
                hps1[:, 0:8], lhsT=w1it_t[:, k, :], rhs=stage_re[:, k, :],
                start=False, stop=(k == _KCH - 1),
            )

        _PREFETCH = 3  # X pool bufs = _PREFETCH + 1
        for j in range(min(_PREFETCH, n_iter)):
            issue_load(j)

        prev1 = None
        prev2 = None
        for j, (b, k) in enumerate(iters):
                X = xtiles.pop(j)
                fr = X[:, 0, :]
                fi = X[:, 1, :]

                # d = fr^2 + fi^2 over [0:HW); d[HW] = sum(fr) via the spike
                d = work.tile([128, _HWP], f32, tag="d")
                nc.vector._custom_dve(
                    SQ2SPK, out=d, in0=fr, in1=fi, s0=float(_HW)
                )
                # mean(fi) on ACT; the throwaway elementwise output goes into
                # the y tile, which Ln overwrites right after (same engine)
                y = work.tile([128, _HW], f32, tag="y")
                nc.scalar.activation(
                    out=y, in_=X[:, 1, 0:_HW], func=AF.Copy, bias=0.0,
                    scale=1.0 / _HW,
                    accum_out=stage_im[:, k, b : b + 1],
                )
                # y = 1/d on ACT via exp(-ln d); both live in one table set
                nc.scalar.activation(out=y, in_=d[:, 0:_HW], func=AF.Ln)
                nc.scalar.activation(out=y, in_=y, func=AF.Exp, scale=-1.0)

                # stage B of the previous iteration: its Exp dependency has
                # had a full iteration to complete, so this never stalls DVE
                nxt2 = emit_stage_b(prev1) if prev1 is not None else None
                # prefetch: X(j+3) reuses X(j-1)'s buffer, whose LAST reader
                # (the gather in stage B above) is now emitted — issuing the
                # load here gives it a tracked WAR dependency on that gather.
                # Issuing any earlier would race the gather's read.
                if j + _PREFETCH < n_iter:
                    issue_load(j + _PREFETCH)

                nsc = work.tile([128, _HW], f32, tag="nsc")
                nc.vector._custom_dve(
                    CSC, out=nsc, in0=d[:, 0:_HW], in1=X[:, 0, 0:_HW], s0=2.0
                )
                # mean(fr) = spike / HW
                nc.gpsimd.tensor_scalar(
                    out=stage_re[:, k, b : b + 1], in0=d[:, _HW:_HWP],
                    scalar1=1.0 / _HW, scalar2=0.0, op0=A.mult, op1=A.add,
                )
                # the first touch of the mask constants on DVE happens here,
                # after the pipeline is rolling, so it never head-of-line
                # blocks the first SQ2 behind the constant DMAs
                if j == 1:
                    nc.vector.tensor_copy(out=junk32, in_=dmask_r_t)
                    nc.vector.tensor_copy(out=junk32, in_=dmask_i_t)

                if prev2 is not None:
                    emit_stage_c(prev2)
                prev2 = nxt2
                prev1 = {"nsc": nsc, "y": y, "X": X, "d": d, "k": k, "b": b}
        # drain the pipeline
        nxt2 = emit_stage_b(prev1)
        if prev2 is not None:
            emit_stage_c(prev2)
        if nxt2 is not None:
            emit_stage_c(nxt2)

        # --- tiny complex MLP, second half (first-layer matmuls were
        # interleaved into the loop per channel chunk) ---
        if repeat != 1:
            for k in range(_KCH):
                emit_l1_matmuls(k)
        hreT = mlp.tile([32, 8], f32)
        nc.vector.tensor_scalar(
            out=hreT, in0=hps0[:, 0:8], scalar1=b1re_t, scalar2=None, op0=A.add
        )
        himT = mlp.tile([32, 8], f32)
        nc.vector.tensor_scalar(
            out=himT, in0=hps1[:, 0:8], scalar1=b1im_t, scalar2=None, op0=A.add
        )

        # cardioid: s = 0.5 * (1 + re / |h|); the SQ2SPK spike position is
        # beyond this 8-element stream, so it acts as a plain a^2 + b^2
        q2 = mlp.tile([32, 8], f32)
        nc.vector._custom_dve(SQ2SPK, out=q2, in0=hreT, in1=himT, s0=float(_HW))
        ah = mlp.tile([32, 8], f32)
        nc.scalar.activation(out=ah, in_=q2, func=AF.Sqrt)
        rh = mlp.tile([32, 8], f32)
        nc.vector.reciprocal(out=rh, in_=ah)
        s = mlp.tile([32, 8], f32)
        nc.vector.tensor_tensor(out=s, in0=hreT, in1=rh, op=A.mult)
        nc.vector.tensor_scalar(out=s, in0=s, scalar1=0.5, scalar2=0.5, op0=A.mult, op1=A.add)
        greT = mlp.tile([32, 8], f32)
        nc.vector.tensor_tensor(out=greT, in0=hreT, in1=s, op=A.mult)
        gimT = mlp.tile([32, 8], f32)
        nc.vector.tensor_tensor(out=gimT, in0=himT, in1=s, op=A.mult)

        out_sb = singles.tile([_BLOC, _C2], f32)
        for m in range(_KCH):
            sl = slice(m * 128, (m + 1) * 128)
            ore = psum.tile([128, 8], f32, tag="ore")
            nc.tensor.matmul(ore, lhsT=w2rt_t[:, sl], rhs=greT, start=True, stop=False)
            nc.tensor.matmul(ore, lhsT=w2itn_t[:, sl], rhs=gimT, start=False, stop=True)
            osb_re = mlp.tile([128, 8], f32, tag="osb")
            nc.scalar.copy(out=osb_re, in_=ore)
            fre = mlp.tile([128, 4], f32, tag="fre")
            nc.vector.tensor_tensor(out=fre, in0=osb_re[:, 0:4], in1=osb_re[:, 4:8], op=A.add)
            nc.vector.tensor_scalar(
                out=fre, in0=fre, scalar1=b2re2_t[:, m : m + 1], scalar2=None, op0=A.add
            )
            tps = psum.tile([4, 128], f32, tag="tps")
            nc.tensor.transpose(tps, fre, ident_t)
            nc.vector.tensor_copy(out=out_sb[:, sl], in_=tps)

            oim = psum.tile([128, 8], f32, tag="oim")
            nc.tensor.matmul(oim, lhsT=w2it_t[:, sl], rhs=greT, start=True, stop=False)
            nc.tensor.matmul(oim, lhsT=w2rt_t[:, sl], rhs=gimT, start=False, stop=True)
            osb_im = mlp.tile([128, 8], f32, tag="osb")
            nc.scalar.copy(out=osb_im, in_=oim)
            fim = mlp.tile([128, 4], f32, tag="fim")
            nc.vector.tensor_tensor(out=fim, in0=osb_im[:, 0:4], in1=osb_im[:, 4:8], op=A.add)
            nc.vector.tensor_scalar(
                out=fim, in0=fim, scalar1=b2im2_t[:, m : m + 1], scalar2=None, op0=A.add
            )
            tps2 = psum.tile([4, 128], f32, tag="tps")
            nc.tensor.transpose(tps2, fim, ident_t)
            nc.vector.tensor_copy(out=out_sb[:, _C + m * 128 : _C + (m + 1) * 128], in_=tps2)

        nc.gpsimd.dma_start(out=out[:], in_=out_sb)

    nc.compile()
    return nc


def _host_inputs(w1r, b1r, w1i, b1i, w2r, b2r, w2i, b2i):
    f32 = np.float32
    shared = {
        "w1rt": np.ascontiguousarray(w1r.T, dtype=f32),
        "w1it": np.ascontiguousarray(w1i.T, dtype=f32),
        "w1itn": np.ascontiguousarray(-w1i.T, dtype=f32),
        "w2rt": np.ascontiguousarray(w2r.T, dtype=f32),
        "w2it": np.ascontiguousarray(w2i.T, dtype=f32),
        "w2itn": np.ascontiguousarray(-w2i.T, dtype=f32),
        "b1re": np.ascontiguousarray((b1r - b1i).reshape(32, 1), dtype=f32),
        "b1im": np.ascontiguousarray((b1r + b1i).reshape(32, 1), dtype=f32),
        "b2re2": np.ascontiguousarray((2.0 * (b2r - b2i)).reshape(_KCH, 128), dtype=f32),
        "b2im2": np.ascontiguousarray((2.0 * (b2r + b2i)).reshape(_KCH, 128), dtype=f32),
        "ident": np.eye(128, dtype=f32),
    }
    p = np.arange(128) % 16
    dm_r = np.zeros((128, 32), dtype=f32)
    dm_r[np.arange(128), p] = 1.0
    dm_i = np.zeros((128, 32), dtype=f32)
    dm_i[np.arange(128), 16 + p] = 1.0
    shared["dmask_r"] = dm_r
    shared["dmask_i"] = dm_i
    return shared


def kernel(x, w1r, b1r, w1i, b1i, w2r, b2r, w2i, b2i):
    global last_results
    from concourse.bass_utils import run_bass_kernel_spmd

    x = np.ascontiguousarray(np.asarray(x), dtype=np.float32)
    args = [np.asarray(a, dtype=np.float32) for a in (w1r, b1r, w1i, b1i, w2r, b2r, w2i, b2i)]
    w1r, b1r, w1i, b1i, w2r, b2r, w2i, b2i = args

    if "nc" not in _STATE:
        _STATE["nc"] = _build_nc()
    nc = _STATE["nc"]

    shared = _host_inputs(w1r, b1r, w1i, b1i, w2r, b2r, w2i, b2i)
    xr3 = x.reshape(_B, _C2, _HW)
    in_maps = []
    for i in range(_NCORES):
        m = dict(shared)
        m["x"] = np.ascontiguousarray(xr3[i * _BLOC : (i + 1) * _BLOC])
        in_maps.append(m)

    trace = os.environ.get("KERNEL_TRACE", "0") == "1"
    res = run_bass_kernel_spmd(nc, in_maps, core_ids=list(range(_NCORES)), trace=trace)
    last_results = res
    return np.concatenate([r["out"] for r in res.results], axis=0)
